# revision 7
# baseline (speedup 1.0000x reference)
"""Trainium2 Bass kernel for nn_AttentionTopologyModule (point-cloud kNN attention).

Contract: kernel(**inputs) takes the FULL unsharded inputs (as produced by
setup_inputs) and returns the FULL [B, C, N] output.  Internally the work is
sharded data-parallel over (batch, query-half): 8 cores, each handling 4096
query points of one batch element (candidates = all 8192 points of that batch
element).  The tiny MLP weights are replicated to every core.

Algorithm per core (all model arithmetic on device):
  setup:  load xyz/feats; sq_m = ||x_m||^2/2 via ACT square + DVE reduce;
          point-projection table Qtab[j,:] = [A_j | Bv_j] where
          A = feats@W1n.T + xyz@W1x.T  (attn branch, neighbor part)
          Bv = feats@Wvn.T + xyz@Wvx.T (value branch, neighbor part)
  P1 (per 128-query tile):
      nd[q,m] = x_q . x_m - ||x_m||^2/2   (PE matmul, f32; row-equivalent
                ordering to -distance)  -> top-16 via max8/max_index/
                match_replace/max8/max_index (exact f32 selection)
      gather Qtab rows by idx (indirect DMA), HV = gathered + CN  (CN[q,:] =
      [c_q | d_q] per-query offsets via PE matmul), accumulate per-channel
      sum/sumsq of HV across the tile via PE ones-matmul into PSUM.
  AR1:  AllReduce the BN batch stats (training-mode BatchNorm over the whole
        batch spans all cores); fold scale s into w2 / Wo (relu is positively
        homogeneous, gamma=1>0), fold t/s into the per-query offsets.
  P2 (per tile): re-gather, add CN' (=CN + t/s), relu, logits = h.w2',
        softmax over K=16, weighted sum over K, o = out@(Wo.T*sv)+bo
        (PE transpose + matmul), accumulate o stats.
  AR2:  AllReduce o stats; s_o/t_o.
  P3:   BN+relu on stashed o (ACT per-partition affine), residual add feats,
        DMA out [64, 4096].
"""

import os
import sys

import numpy as np

sys.path.insert(0, "/opt/trn_rl_repo")

import concourse.bacc as bacc
import concourse.bass as bass
import concourse.mybir as mybir
import concourse.tile as tile
from concourse.bass import IndirectOffsetOnAxis

F32 = mybir.dt.float32
U32 = mybir.dt.uint32
ALU = mybir.AluOpType
ACTF = mybir.ActivationFunctionType
AX = mybir.AxisListType

C = 64      # channels
K = 16      # neighbors
H = 64      # hidden dim
EPS = 1e-5
NEG = -1.0e30
SQRT_HALF = 0.7071067811865476


def _b(ap, ins_at, count):
    """Insert a broadcast (step 0) dim into an AP at position ins_at."""
    pat = [list(p) for p in ap.ap]
    pat = pat[:ins_at] + [[0, count]] + pat[ins_at:]
    return bass.AP(tensor=ap.tensor, offset=ap.offset, ap=pat)


def build_nc(N=8192, NQ=4096, n_cores=8, tot_pairs=None, tot_pts=None,
             ablate_topk=False, stt_engine="vector", debug_out=False):
    """Build the per-core Bass program (SPMD: same program, per-core inputs)."""
    NT = NQ // 128          # query tiles
    NA = N // 128           # point tiles (tables)
    if tot_pairs is None:
        tot_pairs = n_cores * NQ * K    # elements per channel in BN1/BNv stats
    if tot_pts is None:
        tot_pts = n_cores * NQ          # elements per channel in BNo stats

    nc = bacc.Bacc("TRN2", target_bir_lowering=False, debug=False,
                   num_devices=n_cores)

    xyzc = nc.dram_tensor("xyzc", [3, N], F32, kind="ExternalInput")
    xyzq = nc.dram_tensor("xyzq", [4, NQ], F32, kind="ExternalInput")
    xyzT = nc.dram_tensor("xyzT", [N, 3], F32, kind="ExternalInput")
    fc = nc.dram_tensor("fc", [C, N], F32, kind="ExternalInput")
    RABd = nc.dram_tensor("RAB", [C + 3, 2 * C], F32, kind="ExternalInput")
    RCNd = nc.dram_tensor("RCN", [C + 4, 2 * C], F32, kind="ExternalInput")
    w2d = nc.dram_tensor("w2rep", [128, H], F32, kind="ExternalInput")
    gbd = nc.dram_tensor("gb", [2, 2 * C], F32, kind="ExternalInput")
    ROd = nc.dram_tensor("RO", [C + 1, C], F32, kind="ExternalInput")
    gbod = nc.dram_tensor("gbo", [C, 2], F32, kind="ExternalInput")
    outd = nc.dram_tensor("out", [C, NQ], F32, kind="ExternalOutput")
    NTq = NQ // 128
    if debug_out:
        dbg_idx = nc.dram_tensor("dbg_idx", [128, NTq * K], U32, kind="ExternalOutput")
        dbg_stats = nc.dram_tensor("dbg_stats", [2, 4 * C], F32, kind="ExternalOutput")
        dbg_sq = nc.dram_tensor("dbg_sq", [1, N], F32, kind="ExternalOutput")
        dbg_sv = nc.dram_tensor("dbg_sv", [6, 2 * C], F32, kind="ExternalOutput")
        dbg_nd0 = nc.dram_tensor("dbg_nd0", [128, 1024], F32, kind="ExternalOutput")
        dbg_g0 = nc.dram_tensor("dbg_g0", [128, K * 2 * C], F32, kind="ExternalOutput")
        dbg_p2 = nc.dram_tensor("dbg_p2", [128, K * 2 * C + 2 * K + C + 2 * C], F32, kind="ExternalOutput")
        dbg_cn0 = nc.dram_tensor("dbg_cn0", [128, 2 * C], F32, kind="ExternalOutput")
        dbg_ot = nc.dram_tensor("dbg_ot", [C, 2 * 2], F32, kind="ExternalOutput")
        dbg_shb = nc.dram_tensor("dbg_shb", [128, 4 * C], F32, kind="ExternalOutput")
        dbg_sqh = nc.dram_tensor("dbg_sqh", [128, 2 * C], F32, kind="ExternalOutput")

    with tile.TileContext(nc) as tc:
        import contextlib
        ctx = contextlib.ExitStack()
        with ctx:
            sing = ctx.enter_context(tc.tile_pool(name="sing", bufs=1))
            dram = ctx.enter_context(tc.tile_pool(name="dram", bufs=1, space="DRAM"))
            ndp = ctx.enter_context(tc.tile_pool(name="ndp", bufs=2))
            gp = ctx.enter_context(tc.tile_pool(name="gp", bufs=2))
            scrp = ctx.enter_context(tc.tile_pool(name="scrp", bufs=2))
            cnp_sb = ctx.enter_context(tc.tile_pool(name="cnsb", bufs=2))
            shp = ctx.enter_context(tc.tile_pool(name="shp", bufs=3))
            smp = ctx.enter_context(tc.tile_pool(name="smp", bufs=4))
            otp_sb = ctx.enter_context(tc.tile_pool(name="otsb", bufs=2))
            # PSUM pools (8 banks total): nd 2x2 banks, cn 1, stats 1, tr 1, oT 1
            ndps = ctx.enter_context(tc.tile_pool(name="ndps", bufs=2, space="PSUM"))
            cnps = ctx.enter_context(tc.tile_pool(name="cnps", bufs=1, space="PSUM"))
            stps = ctx.enter_context(tc.tile_pool(name="stps", bufs=1, space="PSUM"))
            trps = ctx.enter_context(tc.tile_pool(name="trps", bufs=1, space="PSUM"))
            otps = ctx.enter_context(tc.tile_pool(name="otps", bufs=1, space="PSUM"))

            # ---------------- setup ----------------
            F_sbq = sing.tile([C, NQ], F32)
            nc.sync.dma_start(out=F_sbq, in_=fc[:, 0:NQ])
            C4 = sing.tile([4, N], F32)
            nc.sync.dma_start(out=C4[0:3, :], in_=xyzc[:, :])
            Q4 = sing.tile([4, NQ], F32)
            nc.sync.dma_start(out=Q4, in_=xyzq[:, :])
            XT = sing.tile([128, NA, 3], F32)
            nc.sync.dma_start(out=XT, in_=xyzT[:, :].rearrange("(a p) d -> p a d", p=128))
            RABa = sing.tile([C, 2 * C], F32)
            nc.sync.dma_start(out=RABa, in_=RABd[0:C, :])
            RABx = sing.tile([3, 2 * C], F32)
            nc.sync.dma_start(out=RABx, in_=RABd[C:C + 3, :])
            RCNa = sing.tile([C, 2 * C], F32)
            nc.sync.dma_start(out=RCNa, in_=RCNd[0:C, :])
            RCNx = sing.tile([3, 2 * C], F32)
            nc.sync.dma_start(out=RCNx, in_=RCNd[C:C + 3, :])
            RCNb = sing.tile([1, 2 * C], F32)
            nc.sync.dma_start(out=RCNb, in_=RCNd[C + 3:C + 4, :])
            # negate the xyz rows: c_n = G1c - G1x + b1, d_n = bv - Gvx
            nc.vector.tensor_scalar_mul(RCNx, RCNx, -1.0)
            w2rep = sing.tile([128, H], F32)
            nc.sync.dma_start(out=w2rep, in_=w2d[:, :])
            gRow = sing.tile([1, 2 * C], F32)
            nc.sync.dma_start(out=gRow, in_=gbd[0:1, :])
            bRow = sing.tile([1, 2 * C], F32)
            nc.sync.dma_start(out=bRow, in_=gbd[1:2, :])
            RO = sing.tile([C + 1, C], F32)
            nc.sync.dma_start(out=RO, in_=ROd[:, :])
            gbo = sing.tile([C, 2], F32)
            nc.sync.dma_start(out=gbo, in_=gbod[:, :])
            ones1 = sing.tile([1, 128], F32)
            nc.vector.memset(ones1, 1.0)
            ones128 = sing.tile([128, 1], F32)
            nc.vector.memset(ones128, 1.0)
            # identity for PE transpose
            identI = sing.tile([128, 128], mybir.dt.int32)
            nc.gpsimd.iota(identI, pattern=[[1, 128]], base=0, channel_multiplier=-1)
            ident = sing.tile([128, 128], F32)
            nc.vector.tensor_scalar(ident, identI, 0.0, scalar2=None, op0=ALU.is_equal)

            # sq/2 of candidate points -> row 3 of C4
            XTsq = sing.tile([128, NA * 3], F32)
            nc.scalar.activation(XTsq, XT.rearrange("p a d -> p (a d)"),
                                 ACTF.Square, scale=SQRT_HALF)
            SQ2 = sing.tile([128, NA], F32)
            nc.vector.tensor_reduce(out=SQ2, in_=XTsq.rearrange("p (a d) -> p a d", d=3),
                                    axis=AX.X, op=ALU.add)
            sqd = dram.tile([128, NA], F32)
            nc.sync.dma_start(out=sqd, in_=SQ2)
            nc.sync.dma_start(out=C4[3:4, :].rearrange("o (a p) -> o a p", p=128),
                              in_=sqd[:, :].rearrange("p a -> a p"))

            # point projection tables -> DRAM Qtab [N, 128]
            Qtab = dram.tile([N, 2 * C], F32)
            for a in range(NA):
                pt = slice(a * 128, (a + 1) * 128)
                fstr = shp.tile([C, 128], F32, tag="fstr")
                nc.sync.dma_start(out=fstr, in_=fc[:, pt])
                ps = cnps.tile([128, 2 * C], F32, tag="cps")
                nc.tensor.matmul(ps, lhsT=fstr, rhs=RABa,
                                 start=True, stop=False)
                nc.tensor.matmul(ps, lhsT=C4[0:3, pt], rhs=RABx,
                                 start=False, stop=True)
                tsb = cnp_sb.tile([128, 2 * C], F32)
                nc.scalar.copy(tsb, ps)
                nc.sync.dma_start(out=Qtab[pt, :], in_=tsb)

            idxall = sing.tile([128, NT * K], U32)
            hv_d = dram.tile([NQ, K * 2 * C], F32)
            stat_ps = stps.tile([1, 2 * 2 * C], F32)   # [sum(h|v) | sumsq(h|v)]

            NB2 = N // 1024   # nd psum tiles per query tile

            # ---------------- P1: kNN + BN stats ----------------
            for t in range(NT):
                qs = slice(t * 128, (t + 1) * 128)
                nd = ndp.tile([128, N], F32)
                for b2 in range(NB2):
                    ps = ndps.tile([128, 1024], F32)
                    for hh in range(2):
                        cs = slice(b2 * 1024 + hh * 512, b2 * 1024 + (hh + 1) * 512)
                        nc.tensor.matmul(ps[:, hh * 512:(hh + 1) * 512],
                                         lhsT=Q4[:, qs], rhs=C4[:, cs],
                                         start=True, stop=True)
                    nc.scalar.copy(nd[:, b2 * 1024:(b2 + 1) * 1024], ps)
                if debug_out and t == 0:
                    nc.sync.dma_start(out=dbg_nd0[:, :], in_=nd[:, 0:1024])
                # exact top-16 (5 passes)
                v8a = smp.tile([128, 8], F32)
                v8b = smp.tile([128, 8], F32)
                if ablate_topk:
                    nc.vector.max(out=v8a, in_=nd[:, 0:64])
                    nc.vector.max_index(out=idxall[:, t * K:t * K + 8], in_max=v8a,
                                        in_values=nd[:, 0:64])
                    nc.vector.max_index(out=idxall[:, t * K + 8:t * K + 16],
                                        in_max=v8a, in_values=nd[:, 0:64])
                else:
                    nc.vector.max(out=v8a, in_=nd)
                    nc.vector.max_index(out=idxall[:, t * K:t * K + 8], in_max=v8a, in_values=nd)
                    nc.vector.match_replace(out=nd, in_to_replace=v8a, in_values=nd,
                                            imm_value=NEG)
                    nc.vector.max(out=v8b, in_=nd)
                    nc.vector.max_index(out=idxall[:, t * K + 8:t * K + 16], in_max=v8b,
                                        in_values=nd)
                # CN = [c_q | d_q]
                cps = cnps.tile([128, 2 * C], F32)
                nc.tensor.matmul(cps, lhsT=F_sbq[:, qs], rhs=RCNa, start=True, stop=False)
                nc.tensor.matmul(cps, lhsT=Q4[0:3, qs], rhs=RCNx, start=False, stop=False)
                nc.tensor.matmul(cps, lhsT=ones1, rhs=RCNb, start=False, stop=True)
                cn = cnp_sb.tile([128, 2 * C], F32)
                nc.scalar.copy(cn, cps)
                # gather + add CN -> HV = [h_pre | v_pre]
                G = gp.tile([128, K, 2 * C], F32, tag="g")
                for kk in range(K):
                    nc.gpsimd.indirect_dma_start(
                        out=G[:, kk, :], out_offset=None, in_=Qtab[:, :],
                        in_offset=IndirectOffsetOnAxis(
                            ap=idxall[:, t * K + kk:t * K + kk + 1], axis=0))
                nc.gpsimd.tensor_tensor(out=G, in0=G, in1=_b(cn[:, :], 1, K),
                                        op=ALU.add)
                # stats: per-channel sum & sumsq over (q, k)
                if debug_out and t == 0:
                    nc.sync.dma_start(out=dbg_g0[:, :],
                                      in_=G.rearrange("p k c -> p (k c)"))
                    nc.sync.dma_start(out=dbg_cn0[:, :], in_=cn)
                sqh = scrp.tile([128, K * 2 * C], F32, tag="hvn")
                nc.scalar.activation(sqh, G.rearrange("p k c -> p (k c)"), ACTF.Square)
                SHB = shp.tile([128, 4 * C], F32)
                nc.vector.tensor_reduce(out=SHB[:, 0:2 * C],
                                        in_=G.rearrange("p k c -> p c k"),
                                        axis=AX.X, op=ALU.add)
                nc.vector.tensor_reduce(out=SHB[:, 2 * C:4 * C],
                                        in_=sqh.rearrange("p (k c) -> p c k", k=K),
                                        axis=AX.X, op=ALU.add)
                if debug_out and t == 0:
                    nc.sync.dma_start(out=dbg_shb[:, :], in_=SHB)
                    nc.sync.dma_start(out=dbg_sqh[:, :], in_=sqh.rearrange(
                        "p (k c) -> p k c", k=K)[:, 0, :])
                nc.tensor.matmul(stat_ps, lhsT=ones128, rhs=SHB,
                                 start=(t == 0), stop=(t == NT - 1))
                nc.sync.dma_start(out=hv_d[t * 128:(t + 1) * 128, :],
                                  in_=G.rearrange("p k c -> p (k c)"))

            # ---------------- AR1 ----------------
            stats_sb = sing.tile([1, 4 * C], F32)
            nc.vector.tensor_copy(stats_sb, stat_ps)
            bi1 = dram.tile([1, 4 * C], F32)
            bo1 = dram.tile([1, 4 * C], F32)
            nc.sync.dma_start(out=bi1, in_=stats_sb)
            if n_cores > 1:
                nc.gpsimd.collective_compute(
                    "AllReduce", ALU.add,
                    replica_groups=[list(range(n_cores))],
                    ins=[bi1[:, :].opt()], outs=[bo1[:, :].opt()])
            else:
                nc.sync.dma_start(out=bo1[:, :], in_=bi1[:, :])
            stats2 = sing.tile([1, 4 * C], F32)
            nc.sync.dma_start(out=stats2, in_=bo1)

            mean = sing.tile([1, 2 * C], F32)
            nc.vector.tensor_scalar_mul(mean, stats2[:, 0:2 * C], 1.0 / tot_pairs)
            var = sing.tile([1, 2 * C], F32)
            nc.vector.tensor_scalar_mul(var, stats2[:, 2 * C:4 * C], 1.0 / tot_pairs)
            msq = sing.tile([1, 2 * C], F32)
            nc.vector.tensor_mul(msq, mean, mean)
            nc.vector.tensor_sub(var, var, msq)
            nc.vector.tensor_scalar_add(var, var, EPS)
            sdv = sing.tile([1, 2 * C], F32)
            nc.scalar.sqrt(sdv, var)
            rstd = sing.tile([1, 2 * C], F32)
            nc.vector.reciprocal(rstd, sdv)
            svec = sing.tile([1, 2 * C], F32)
            nc.vector.tensor_mul(svec, gRow, rstd)
            tvec = sing.tile([1, 2 * C], F32)
            nc.vector.tensor_mul(tvec, mean, svec)
            nc.vector.tensor_sub(tvec, bRow, tvec)
            sinv = sing.tile([1, 2 * C], F32)
            nc.vector.reciprocal(sinv, svec)
            tps_row = sing.tile([1, 2 * C], F32)   # t/s row for CN'
            nc.vector.tensor_mul(tps_row, tvec, sinv)
            # replicate s_h across partitions via PE rank-1 broadcast
            # (0-stride partition DMA is not supported by the hardware DGE)
            srep_ps = cnps.tile([128, H], F32, tag="cps")
            nc.tensor.matmul(srep_ps, lhsT=ones1, rhs=svec[:, 0:C],
                             start=True, stop=True)
            srep = sing.tile([128, H], F32)
            nc.scalar.copy(srep, srep_ps)
            sdr = dram.tile([1, 2 * C], F32)
            nc.sync.dma_start(out=sdr, in_=svec)
            sv64 = sing.tile([C, 1], F32)
            nc.sync.dma_start(out=sv64, in_=sdr[0, C:2 * C].rearrange("(p o) -> p o", o=1))
            # fold s into w2 and Wo
            w2p = sing.tile([128, H], F32)
            nc.vector.tensor_mul(w2p, w2rep, srep)
            ROp = sing.tile([C + 1, C], F32)
            nc.vector.tensor_mul(ROp[0:C, :], RO[0:C, :], sv64.to_broadcast([C, C]))
            nc.vector.tensor_copy(ROp[C:C + 1, :], RO[C:C + 1, :])

            if debug_out:
                nc.sync.dma_start(out=dbg_idx[:, :], in_=idxall)
                nc.sync.dma_start(out=dbg_sq[:, :], in_=C4[3:4, :])
                nc.sync.dma_start(out=dbg_stats[0:1, :], in_=stats_sb)
                nc.sync.dma_start(out=dbg_stats[1:2, :], in_=stats2)
                nc.sync.dma_start(out=dbg_sv[0:1, :], in_=svec)
                nc.sync.dma_start(out=dbg_sv[1:2, :], in_=tvec)
                nc.sync.dma_start(out=dbg_sv[2:3, 0:C], in_=srep[100:101, :])
                nc.sync.dma_start(out=dbg_sv[3:4, 0:C], in_=w2p[100:101, :])
                nc.sync.dma_start(out=dbg_sv[4:5, 0:C].rearrange("o (p q) -> p (o q)", q=1), in_=sv64[:, 0:1])
                nc.sync.dma_start(out=dbg_sv[5:6, :], in_=tps_row)
            t128_ps = cnps.tile([128, 2 * C], F32, tag="cps")
            nc.tensor.matmul(t128_ps, lhsT=ones1, rhs=tps_row, start=True, stop=True)
            t128 = sing.tile([128, 2 * C], F32)
            nc.scalar.copy(t128, t128_ps)
            ostash_d = dram.tile([C, NQ], F32)
            osums = sing.tile([C, NT], F32)
            osums2 = sing.tile([C, NT], F32)

            # ---------------- P2: attention + value + output proj ----------------
            for t in range(NT):
                qs = slice(t * 128, (t + 1) * 128)
                G2 = gp.tile([128, K, 2 * C], F32, tag="g")
                nc.sync.dma_start(out=G2,
                                  in_=hv_d[t * 128:(t + 1) * 128, :].rearrange(
                                      "p (k c) -> p k c", k=K))
                nc.gpsimd.tensor_tensor(out=G2, in0=G2, in1=_b(t128[:, :], 1, K),
                                        op=ALU.add)
                HVn = scrp.tile([128, K * 2 * C], F32, tag="hvn")
                nc.scalar.activation(HVn, G2.rearrange("p k c -> p (k c)"), ACTF.Relu)
                HVn3 = HVn.rearrange("p (k c) -> p k c", k=K)
                # logits & softmax over K
                lsc = scrp.tile([128, K, H], F32, tag="lsc")
                getattr(nc, stt_engine).scalar_tensor_tensor(
                    out=lsc, in0=HVn3[:, :, 0:C], scalar=0.0,
                    in1=_b(w2p[:, :], 1, K), op0=ALU.bypass, op1=ALU.mult)
                logit = smp.tile([128, K], F32)
                nc.vector.tensor_reduce(out=logit, in_=lsc, axis=AX.X, op=ALU.add)
                mx = smp.tile([128, 1], F32)
                nc.vector.tensor_reduce(out=mx, in_=logit, axis=AX.X, op=ALU.max)
                nc.vector.tensor_scalar_mul(mx, mx, -1.0)
                ex = smp.tile([128, K], F32)
                nc.scalar.activation(ex, logit, ACTF.Exp, bias=mx[:, 0:1])
                sume = smp.tile([128, 1], F32)
                nc.vector.tensor_reduce(out=sume, in_=ex, axis=AX.X, op=ALU.add)
                rec = smp.tile([128, 1], F32)
                nc.vector.reciprocal(rec, sume)
                attn = smp.tile([128, K], F32)
                nc.vector.tensor_scalar_mul(attn, ex, rec[:, 0:1])
                # weighted sum over K
                prod = scrp.tile([128, K, C], F32, tag="lsc")
                getattr(nc, stt_engine).scalar_tensor_tensor(
                    out=prod, in0=HVn3[:, :, C:2 * C], scalar=0.0,
                    in1=_b(attn[:, :], 2, C), op0=ALU.bypass, op1=ALU.mult)
                outq = smp.tile([128, C], F32, tag="outq")
                nc.vector.tensor_reduce(out=outq, in_=prod.rearrange("p k c -> p c k"),
                                        axis=AX.X, op=ALU.add)
                if debug_out and t == 0:
                    nc.sync.dma_start(out=dbg_p2[:, 0:K * 2 * C], in_=HVn)
                    nc.sync.dma_start(out=dbg_p2[:, K * 2 * C:K * 2 * C + K], in_=logit)
                    nc.sync.dma_start(out=dbg_p2[:, K * 2 * C + K:K * 2 * C + 2 * K], in_=attn)
                    nc.sync.dma_start(out=dbg_p2[:, K * 2 * C + 2 * K:K * 2 * C + 2 * K + C], in_=outq)
                    nc.sync.dma_start(out=dbg_p2[:, K * 2 * C + 2 * K + C:], in_=t128)
                # o = (out @ Wo.T * sv) + bo, via transpose + matmul
                tps = trps.tile([C, 128], F32)
                nc.tensor.transpose(tps, outq, ident)
                ot5 = otp_sb.tile([C + 1, 128], F32)
                nc.vector.memset(ot5[C:C + 1, :], 1.0)
                nc.scalar.copy(ot5[0:C, :], tps)
                ops_ = otps.tile([C, 128], F32)
                nc.tensor.matmul(ops_, lhsT=ROp, rhs=ot5, start=True, stop=True)
                osb = otp_sb.tile([C, 128], F32, tag="osb")
                nc.scalar.activation(osb, ops_, ACTF.Copy,
                                     accum_out=osums[:, t:t + 1])
                nc.sync.dma_start(out=ostash_d[:, qs], in_=osb)
                osq = otp_sb.tile([C, 128], F32, tag="osq")
                nc.scalar.activation(osq, ops_, ACTF.Square,
                                     accum_out=osums2[:, t:t + 1])

            # ---------------- AR2 ----------------
            ost = sing.tile([C, 2], F32)
            nc.vector.tensor_reduce(out=ost[:, 0:1], in_=osums, axis=AX.X, op=ALU.add)
            nc.vector.tensor_reduce(out=ost[:, 1:2], in_=osums2, axis=AX.X, op=ALU.add)
            bi2 = dram.tile([C, 2], F32)
            bo2 = dram.tile([C, 2], F32)
            nc.sync.dma_start(out=bi2, in_=ost)
            if n_cores > 1:
                nc.gpsimd.collective_compute(
                    "AllReduce", ALU.add,
                    replica_groups=[list(range(n_cores))],
                    ins=[bi2[:, :].opt()], outs=[bo2[:, :].opt()])
            else:
                nc.sync.dma_start(out=bo2[:, :], in_=bi2[:, :])
            ost2 = sing.tile([C, 2], F32)
            nc.sync.dma_start(out=ost2, in_=bo2)
            omean = sing.tile([C, 1], F32)
            nc.vector.tensor_scalar_mul(omean, ost2[:, 0:1], 1.0 / tot_pts)
            ovar = sing.tile([C, 1], F32)
            nc.vector.tensor_scalar_mul(ovar, ost2[:, 1:2], 1.0 / tot_pts)
            omsq = sing.tile([C, 1], F32)
            nc.vector.tensor_mul(omsq, omean, omean)
            nc.vector.tensor_sub(ovar, ovar, omsq)
            nc.vector.tensor_scalar_add(ovar, ovar, EPS)
            osd = sing.tile([C, 1], F32)
            nc.scalar.sqrt(osd, ovar)
            orst = sing.tile([C, 1], F32)
            nc.vector.reciprocal(orst, osd)
            so = sing.tile([C, 1], F32)
            nc.vector.tensor_mul(so, gbo[:, 0:1], orst)
            to = sing.tile([C, 1], F32)
            nc.vector.tensor_mul(to, omean, so)
            nc.vector.tensor_sub(to, gbo[:, 1:2], to)

            if debug_out:
                nc.sync.dma_start(out=dbg_ot[:, 0:1], in_=so)
                nc.sync.dma_start(out=dbg_ot[:, 1:2], in_=to)
                nc.sync.dma_start(out=dbg_ot[:, 2:3], in_=ost[:, 0:1])
                nc.sync.dma_start(out=dbg_ot[:, 3:4], in_=ost2[:, 0:1])
            # ---------------- P3: BN + relu + residual ----------------
            P3CH = min(1024, NQ)
            for j in range(NQ // P3CH):
                js = slice(j * P3CH, (j + 1) * P3CH)
                ob = scrp.tile([C, P3CH], F32, tag="hvn")
                nc.sync.dma_start(out=ob, in_=ostash_d[:, js])
                nc.scalar.activation(ob, ob, ACTF.Relu, bias=to[:, 0:1],
                                     scale=so[:, 0:1])
                nc.vector.scalar_tensor_tensor(out=ob, in0=ob, scalar=0.0,
                                               in1=F_sbq[:, js],
                                               op0=ALU.bypass, op1=ALU.add)
                nc.sync.dma_start(out=outd[:, js], in_=ob)

    nc.compile()
    return nc


def make_in_maps(xyz, feats, W1, b1, g1, be1, W2, b2, Wv, bv, gv, bev,
                 Wo, bo, go, beo, n_cores=8, N=8192, NQ=4096):
    """Shard/lay out the full inputs into per-core input dicts (layout only)."""
    f32 = np.float32
    W1 = np.asarray(W1, f32)
    Wv = np.asarray(Wv, f32)
    # RAB: rows [in-ch(64); xyz(3)], cols [A(64) | Bv(64)]
    RAB = np.concatenate([
        np.concatenate([W1[:, C:2 * C].T, W1[:, 2 * C:2 * C + 3].T], axis=0),
        np.concatenate([Wv[:, 0:C].T, Wv[:, C:C + 3].T], axis=0),
    ], axis=1).astype(f32)
    RCN = np.concatenate([
        np.concatenate([W1[:, 0:C].T, W1[:, 2 * C:2 * C + 3].T,
                        np.asarray(b1, f32)[None, :]], axis=0),
        np.concatenate([np.zeros((C, C), f32), Wv[:, C:C + 3].T,
                        np.asarray(bv, f32)[None, :]], axis=0),
    ], axis=1).astype(f32)
    w2rep = np.ascontiguousarray(np.broadcast_to(np.asarray(W2, f32)[0], (128, H)))
    gbp = np.stack([np.concatenate([np.asarray(g1, f32), np.asarray(gv, f32)]),
                    np.concatenate([np.asarray(be1, f32), np.asarray(bev, f32)])])
    RO = np.concatenate([np.asarray(Wo, f32).T, np.asarray(bo, f32)[None, :]], axis=0)
    gbo = np.stack([np.asarray(go, f32), np.asarray(beo, f32)], axis=1)

    xyz = np.asarray(xyz, f32)
    feats = np.asarray(feats, f32)
    halves = n_cores // xyz.shape[0]      # cores per batch element
    in_maps = []
    for c in range(n_cores):
        b = c // halves
        h = c % halves
        xb = np.roll(xyz[b], -h * NQ, axis=1)
        fb = np.roll(feats[b], -h * NQ, axis=1)
        in_maps.append({
            "xyzc": np.ascontiguousarray(xb),
            "xyzq": np.ascontiguousarray(
                np.concatenate([xb[:, 0:NQ], -np.ones((1, NQ), f32)], axis=0)),
            "xyzT": np.ascontiguousarray(xb.T),
            "fc": np.ascontiguousarray(fb),
            "RAB": RAB, "RCN": RCN, "w2rep": w2rep, "gb": gbp,
            "RO": np.ascontiguousarray(RO), "gbo": np.ascontiguousarray(gbo),
        })
    return in_maps


_NC_CACHE = {}


def kernel(**inputs):
    from concourse.bass_utils import run_bass_kernel_spmd
    B, _, N = inputs["xyz"].shape
    n_cores = 8
    NQ = N * B // n_cores
    key = (N, NQ, n_cores)
    if key not in _NC_CACHE:
        _NC_CACHE[key] = build_nc(N=N, NQ=NQ, n_cores=n_cores)
    nc = _NC_CACHE[key]
    in_maps = make_in_maps(n_cores=n_cores, N=N, NQ=NQ, **inputs)
    res = run_bass_kernel_spmd(nc, in_maps, core_ids=list(range(n_cores)))
    halves = n_cores // B
    out = np.empty((B, C, N), np.float32)
    for c in range(n_cores):
        b, h = c // halves, c % halves
        out[b][:, h * NQ:(h + 1) * NQ] = res.results[c]["out"]
    return out



# revision 9
# speedup vs baseline: 1.2765x; 1.2765x over previous
"""Trainium2 Bass kernel for nn_AttentionTopologyModule (point-cloud kNN attention).

Contract: kernel(**inputs) takes the FULL unsharded inputs (as produced by
setup_inputs) and returns the FULL [B, C, N] output.  Internally the work is
sharded data-parallel over (batch, query-half): 8 cores, each handling 4096
query points of one batch element (candidates = all 8192 points of that batch
element).  The tiny MLP weights are replicated to every core.

Algorithm per core (all model arithmetic on device):
  setup:  load xyz/feats; sq_m = ||x_m||^2/2 via ACT square + DVE reduce;
          point-projection table Qtab[j,:] = [A_j | Bv_j] where
          A = feats@W1n.T + xyz@W1x.T  (attn branch, neighbor part)
          Bv = feats@Wvn.T + xyz@Wvx.T (value branch, neighbor part)
  P1 (per 128-query tile):
      nd[q,m] = x_q . x_m - ||x_m||^2/2   (PE matmul, f32; row-equivalent
                ordering to -distance)  -> top-16 via max8/max_index/
                match_replace/max8/max_index (exact f32 selection)
      gather Qtab rows by idx (indirect DMA), HV = gathered + CN  (CN[q,:] =
      [c_q | d_q] per-query offsets via PE matmul), accumulate per-channel
      sum/sumsq of HV across the tile via PE ones-matmul into PSUM.
  AR1:  AllReduce the BN batch stats (training-mode BatchNorm over the whole
        batch spans all cores); fold scale s into w2 / Wo (relu is positively
        homogeneous, gamma=1>0), fold t/s into the per-query offsets.
  P2 (per tile): re-gather, add CN' (=CN + t/s), relu, logits = h.w2',
        softmax over K=16, weighted sum over K, o = out@(Wo.T*sv)+bo
        (PE transpose + matmul), accumulate o stats.
  AR2:  AllReduce o stats; s_o/t_o.
  P3:   BN+relu on stashed o (ACT per-partition affine), residual add feats,
        DMA out [64, 4096].
"""

import os
import sys

import numpy as np

sys.path.insert(0, "/opt/trn_rl_repo")

import concourse.bacc as bacc
import concourse.bass as bass
import concourse.mybir as mybir
import concourse.tile as tile
from concourse.bass import IndirectOffsetOnAxis

F32 = mybir.dt.float32
U32 = mybir.dt.uint32
ALU = mybir.AluOpType
ACTF = mybir.ActivationFunctionType
AX = mybir.AxisListType

C = 64      # channels
K = 16      # neighbors
H = 64      # hidden dim
EPS = 1e-5
NEG = -1.0e30
SQRT_HALF = 0.7071067811865476


def _b(ap, ins_at, count):
    """Insert a broadcast (step 0) dim into an AP at position ins_at."""
    pat = [list(p) for p in ap.ap]
    pat = pat[:ins_at] + [[0, count]] + pat[ins_at:]
    return bass.AP(tensor=ap.tensor, offset=ap.offset, ap=pat)


def build_nc(N=8192, NQ=4096, n_cores=8, tot_pairs=None, tot_pts=None,
             ablate_topk=False, stt_engine="vector", debug_out=False):
    """Build the per-core Bass program (SPMD: same program, per-core inputs)."""
    NT = NQ // 128          # query tiles
    NA = N // 128           # point tiles (tables)
    if tot_pairs is None:
        tot_pairs = n_cores * NQ * K    # elements per channel in BN1/BNv stats
    if tot_pts is None:
        tot_pts = n_cores * NQ          # elements per channel in BNo stats

    nc = bacc.Bacc("TRN2", target_bir_lowering=False, debug=False,
                   num_devices=n_cores)

    xyzc = nc.dram_tensor("xyzc", [3, N], F32, kind="ExternalInput")
    xyzq = nc.dram_tensor("xyzq", [4, NQ], F32, kind="ExternalInput")
    xyzT = nc.dram_tensor("xyzT", [N, 3], F32, kind="ExternalInput")
    fc = nc.dram_tensor("fc", [C, N], F32, kind="ExternalInput")
    RABd = nc.dram_tensor("RAB", [C + 3, 2 * C], F32, kind="ExternalInput")
    RCNd = nc.dram_tensor("RCN", [C + 4, 2 * C], F32, kind="ExternalInput")
    w2d = nc.dram_tensor("w2rep", [128, H], F32, kind="ExternalInput")
    gbd = nc.dram_tensor("gb", [2, 2 * C], F32, kind="ExternalInput")
    ROd = nc.dram_tensor("RO", [C + 1, C], F32, kind="ExternalInput")
    gbod = nc.dram_tensor("gbo", [C, 2], F32, kind="ExternalInput")
    outd = nc.dram_tensor("out", [C, NQ], F32, kind="ExternalOutput")
    NTq = NQ // 128
    if debug_out:
        dbg_idx = nc.dram_tensor("dbg_idx", [128, NTq * K], U32, kind="ExternalOutput")
        dbg_stats = nc.dram_tensor("dbg_stats", [2, 4 * C], F32, kind="ExternalOutput")
        dbg_sq = nc.dram_tensor("dbg_sq", [1, N], F32, kind="ExternalOutput")
        dbg_sv = nc.dram_tensor("dbg_sv", [6, 2 * C], F32, kind="ExternalOutput")
        dbg_nd0 = nc.dram_tensor("dbg_nd0", [128, 1024], F32, kind="ExternalOutput")
        dbg_g0 = nc.dram_tensor("dbg_g0", [128, K * 2 * C], F32, kind="ExternalOutput")
        dbg_p2 = nc.dram_tensor("dbg_p2", [128, K * 2 * C + 2 * K + C + 2 * C], F32, kind="ExternalOutput")
        dbg_cn0 = nc.dram_tensor("dbg_cn0", [128, 2 * C], F32, kind="ExternalOutput")
        dbg_ot = nc.dram_tensor("dbg_ot", [C, 2 * 2], F32, kind="ExternalOutput")
        dbg_shb = nc.dram_tensor("dbg_shb", [128, 4 * C], F32, kind="ExternalOutput")
        dbg_sqh = nc.dram_tensor("dbg_sqh", [128, 2 * C], F32, kind="ExternalOutput")

    with tile.TileContext(nc) as tc:
        import contextlib
        ctx = contextlib.ExitStack()
        with ctx:
            sing = ctx.enter_context(tc.tile_pool(name="sing", bufs=1))
            dram = ctx.enter_context(tc.tile_pool(name="dram", bufs=1, space="DRAM"))
            ndp = ctx.enter_context(tc.tile_pool(name="ndp", bufs=2))
            gp = ctx.enter_context(tc.tile_pool(name="gp", bufs=2))
            scrp = ctx.enter_context(tc.tile_pool(name="scrp", bufs=2))
            cnp_sb = ctx.enter_context(tc.tile_pool(name="cnsb", bufs=2))
            shp = ctx.enter_context(tc.tile_pool(name="shp", bufs=3))
            smp = ctx.enter_context(tc.tile_pool(name="smp", bufs=4))
            otp_sb = ctx.enter_context(tc.tile_pool(name="otsb", bufs=2))
            # PSUM pools (8 banks total): nd 2x2 banks, cn 1, stats 1, tr 1, oT 1
            ndps = ctx.enter_context(tc.tile_pool(name="ndps", bufs=2, space="PSUM"))
            cnps = ctx.enter_context(tc.tile_pool(name="cnps", bufs=1, space="PSUM"))
            stps = ctx.enter_context(tc.tile_pool(name="stps", bufs=1, space="PSUM"))
            trps = ctx.enter_context(tc.tile_pool(name="trps", bufs=1, space="PSUM"))
            otps = ctx.enter_context(tc.tile_pool(name="otps", bufs=1, space="PSUM"))

            # ---------------- setup ----------------
            F_sbq = sing.tile([C, NQ], F32)
            nc.sync.dma_start(out=F_sbq, in_=fc[:, 0:NQ])
            C4 = sing.tile([4, N], F32)
            nc.sync.dma_start(out=C4[0:3, :], in_=xyzc[:, :])
            Q4 = sing.tile([4, NQ], F32)
            nc.sync.dma_start(out=Q4, in_=xyzq[:, :])
            XT = sing.tile([128, NA, 3], F32)
            nc.sync.dma_start(out=XT, in_=xyzT[:, :].rearrange("(a p) d -> p a d", p=128))
            RABa = sing.tile([C, 2 * C], F32)
            nc.sync.dma_start(out=RABa, in_=RABd[0:C, :])
            RABx = sing.tile([3, 2 * C], F32)
            nc.sync.dma_start(out=RABx, in_=RABd[C:C + 3, :])
            RCNa = sing.tile([C, 2 * C], F32)
            nc.sync.dma_start(out=RCNa, in_=RCNd[0:C, :])
            RCNx = sing.tile([3, 2 * C], F32)
            nc.sync.dma_start(out=RCNx, in_=RCNd[C:C + 3, :])
            RCNb = sing.tile([1, 2 * C], F32)
            nc.sync.dma_start(out=RCNb, in_=RCNd[C + 3:C + 4, :])
            # negate the xyz rows: c_n = G1c - G1x + b1, d_n = bv - Gvx
            nc.vector.tensor_scalar_mul(RCNx, RCNx, -1.0)
            w2rep = sing.tile([128, H], F32)
            nc.sync.dma_start(out=w2rep, in_=w2d[:, :])
            gRow = sing.tile([1, 2 * C], F32)
            nc.sync.dma_start(out=gRow, in_=gbd[0:1, :])
            bRow = sing.tile([1, 2 * C], F32)
            nc.sync.dma_start(out=bRow, in_=gbd[1:2, :])
            RO = sing.tile([C + 1, C], F32)
            nc.sync.dma_start(out=RO, in_=ROd[:, :])
            gbo = sing.tile([C, 2], F32)
            nc.sync.dma_start(out=gbo, in_=gbod[:, :])
            ones1 = sing.tile([1, 128], F32)
            nc.vector.memset(ones1, 1.0)
            ones128 = sing.tile([128, 1], F32)
            nc.vector.memset(ones128, 1.0)
            # identity for PE transpose
            identI = sing.tile([128, 128], mybir.dt.int32)
            nc.gpsimd.iota(identI, pattern=[[1, 128]], base=0, channel_multiplier=-1)
            ident = sing.tile([128, 128], F32)
            nc.vector.tensor_scalar(ident, identI, 0.0, scalar2=None, op0=ALU.is_equal)

            # sq/2 of candidate points -> row 3 of C4
            XTsq = sing.tile([128, NA * 3], F32)
            nc.scalar.activation(XTsq, XT.rearrange("p a d -> p (a d)"),
                                 ACTF.Square, scale=SQRT_HALF)
            SQ2 = sing.tile([128, NA], F32)
            nc.vector.tensor_reduce(out=SQ2, in_=XTsq.rearrange("p (a d) -> p a d", d=3),
                                    axis=AX.X, op=ALU.add)
            sqd = dram.tile([128, NA], F32)
            nc.sync.dma_start(out=sqd, in_=SQ2)
            nc.sync.dma_start(out=C4[3:4, :].rearrange("o (a p) -> o a p", p=128),
                              in_=sqd[:, :].rearrange("p a -> a p"))

            # point projection tables -> DRAM Qtab [N, 128]
            Qtab = dram.tile([N, 2 * C], F32)
            for a in range(NA):
                pt = slice(a * 128, (a + 1) * 128)
                fstr = shp.tile([C, 128], F32, tag="fstr")
                nc.sync.dma_start(out=fstr, in_=fc[:, pt])
                ps = cnps.tile([128, 2 * C], F32, tag="cps")
                nc.tensor.matmul(ps, lhsT=fstr, rhs=RABa,
                                 start=True, stop=False)
                nc.tensor.matmul(ps, lhsT=C4[0:3, pt], rhs=RABx,
                                 start=False, stop=True)
                tsb = cnp_sb.tile([128, 2 * C], F32)
                nc.scalar.copy(tsb, ps)
                nc.sync.dma_start(out=Qtab[pt, :], in_=tsb)

            idxall = sing.tile([128, NT * K], U32)
            hv_d = dram.tile([NQ, K * 2 * C], F32)
            stat_ps = stps.tile([1, 2 * 2 * C], F32)   # [sum(h|v) | sumsq(h|v)]

            NB2 = N // 1024   # nd psum tiles per query tile

            # ---------------- P1: kNN + BN stats ----------------
            for t in range(NT):
                qs = slice(t * 128, (t + 1) * 128)
                nd = ndp.tile([128, N], F32)
                for b2 in range(NB2):
                    ps = ndps.tile([128, 1024], F32)
                    for hh in range(2):
                        cs = slice(b2 * 1024 + hh * 512, b2 * 1024 + (hh + 1) * 512)
                        nc.tensor.matmul(ps[:, hh * 512:(hh + 1) * 512],
                                         lhsT=Q4[:, qs], rhs=C4[:, cs],
                                         start=True, stop=True)
                    nc.scalar.copy(nd[:, b2 * 1024:(b2 + 1) * 1024], ps)
                if debug_out and t == 0:
                    nc.sync.dma_start(out=dbg_nd0[:, :], in_=nd[:, 0:1024])
                # exact top-16 (5 passes)
                v8a = smp.tile([128, 8], F32)
                v8b = smp.tile([128, 8], F32)
                if ablate_topk:
                    nc.vector.max(out=v8a, in_=nd[:, 0:64])
                    nc.vector.max_index(out=idxall[:, t * K:t * K + 8], in_max=v8a,
                                        in_values=nd[:, 0:64])
                    nc.vector.max_index(out=idxall[:, t * K + 8:t * K + 16],
                                        in_max=v8a, in_values=nd[:, 0:64])
                else:
                    nc.vector.max(out=v8a, in_=nd)
                    nc.vector.max_index(out=idxall[:, t * K:t * K + 8], in_max=v8a, in_values=nd)
                    nc.vector.match_replace(out=nd, in_to_replace=v8a, in_values=nd,
                                            imm_value=NEG)
                    nc.vector.max(out=v8b, in_=nd)
                    nc.vector.max_index(out=idxall[:, t * K + 8:t * K + 16], in_max=v8b,
                                        in_values=nd)
                # CN = [c_q | d_q]
                cps = cnps.tile([128, 2 * C], F32)
                nc.tensor.matmul(cps, lhsT=F_sbq[:, qs], rhs=RCNa, start=True, stop=False)
                nc.tensor.matmul(cps, lhsT=Q4[0:3, qs], rhs=RCNx, start=False, stop=False)
                nc.tensor.matmul(cps, lhsT=ones1, rhs=RCNb, start=False, stop=True)
                cn = cnp_sb.tile([128, 2 * C], F32)
                nc.scalar.copy(cn, cps)
                # gather + add CN -> HV = [h_pre | v_pre]
                G = gp.tile([128, K, 2 * C], F32, tag="g")
                for kk in range(K):
                    nc.gpsimd.indirect_dma_start(
                        out=G[:, kk, :], out_offset=None, in_=Qtab[:, :],
                        in_offset=IndirectOffsetOnAxis(
                            ap=idxall[:, t * K + kk:t * K + kk + 1], axis=0))
                getattr(nc, stt_engine).scalar_tensor_tensor(
                    out=G, in0=G, scalar=0.0, in1=_b(cn[:, :], 1, K),
                    op0=ALU.bypass, op1=ALU.add)
                # stats: per-channel sum & sumsq over (q, k)
                if debug_out and t == 0:
                    nc.sync.dma_start(out=dbg_g0[:, :],
                                      in_=G.rearrange("p k c -> p (k c)"))
                    nc.sync.dma_start(out=dbg_cn0[:, :], in_=cn)
                sqh = scrp.tile([128, K * 2 * C], F32, tag="hvn")
                nc.scalar.activation(sqh, G.rearrange("p k c -> p (k c)"), ACTF.Square)
                SHB = shp.tile([128, 4 * C], F32)
                nc.vector.tensor_reduce(out=SHB[:, 0:2 * C],
                                        in_=G.rearrange("p k c -> p c k"),
                                        axis=AX.X, op=ALU.add)
                nc.vector.tensor_reduce(out=SHB[:, 2 * C:4 * C],
                                        in_=sqh.rearrange("p (k c) -> p c k", k=K),
                                        axis=AX.X, op=ALU.add)
                if debug_out and t == 0:
                    nc.sync.dma_start(out=dbg_shb[:, :], in_=SHB)
                    nc.sync.dma_start(out=dbg_sqh[:, :], in_=sqh.rearrange(
                        "p (k c) -> p k c", k=K)[:, 0, :])
                nc.tensor.matmul(stat_ps, lhsT=ones128, rhs=SHB,
                                 start=(t == 0), stop=(t == NT - 1))
                nc.sync.dma_start(out=hv_d[t * 128:(t + 1) * 128, :],
                                  in_=G.rearrange("p k c -> p (k c)"))

            # ---------------- AR1 ----------------
            stats_sb = sing.tile([1, 4 * C], F32)
            nc.vector.tensor_copy(stats_sb, stat_ps)
            bi1 = dram.tile([1, 4 * C], F32)
            bo1 = dram.tile([1, 4 * C], F32)
            nc.sync.dma_start(out=bi1, in_=stats_sb)
            if n_cores > 1:
                nc.gpsimd.collective_compute(
                    "AllReduce", ALU.add,
                    replica_groups=[list(range(n_cores))],
                    ins=[bi1[:, :].opt()], outs=[bo1[:, :].opt()])
            else:
                nc.sync.dma_start(out=bo1[:, :], in_=bi1[:, :])
            stats2 = sing.tile([1, 4 * C], F32)
            nc.sync.dma_start(out=stats2, in_=bo1)

            mean = sing.tile([1, 2 * C], F32)
            nc.vector.tensor_scalar_mul(mean, stats2[:, 0:2 * C], 1.0 / tot_pairs)
            var = sing.tile([1, 2 * C], F32)
            nc.vector.tensor_scalar_mul(var, stats2[:, 2 * C:4 * C], 1.0 / tot_pairs)
            msq = sing.tile([1, 2 * C], F32)
            nc.vector.tensor_mul(msq, mean, mean)
            nc.vector.tensor_sub(var, var, msq)
            nc.vector.tensor_scalar_add(var, var, EPS)
            sdv = sing.tile([1, 2 * C], F32)
            nc.scalar.sqrt(sdv, var)
            rstd = sing.tile([1, 2 * C], F32)
            nc.vector.reciprocal(rstd, sdv)
            svec = sing.tile([1, 2 * C], F32)
            nc.vector.tensor_mul(svec, gRow, rstd)
            tvec = sing.tile([1, 2 * C], F32)
            nc.vector.tensor_mul(tvec, mean, svec)
            nc.vector.tensor_sub(tvec, bRow, tvec)
            sinv = sing.tile([1, 2 * C], F32)
            nc.vector.reciprocal(sinv, svec)
            tps_row = sing.tile([1, 2 * C], F32)   # t/s row for CN'
            nc.vector.tensor_mul(tps_row, tvec, sinv)
            # replicate s_h across partitions via PE rank-1 broadcast
            # (0-stride partition DMA is not supported by the hardware DGE)
            srep_ps = cnps.tile([128, H], F32, tag="cps")
            nc.tensor.matmul(srep_ps, lhsT=ones1, rhs=svec[:, 0:C],
                             start=True, stop=True)
            srep = sing.tile([128, H], F32)
            nc.scalar.copy(srep, srep_ps)
            sdr = dram.tile([1, 2 * C], F32)
            nc.sync.dma_start(out=sdr, in_=svec)
            sv64 = sing.tile([C, 1], F32)
            nc.sync.dma_start(out=sv64, in_=sdr[0, C:2 * C].rearrange("(p o) -> p o", o=1))
            # fold s into w2 and Wo
            w2p = sing.tile([128, H], F32)
            nc.vector.tensor_mul(w2p, w2rep, srep)
            ROp = sing.tile([C + 1, C], F32)
            nc.vector.tensor_mul(ROp[0:C, :], RO[0:C, :], sv64.to_broadcast([C, C]))
            nc.vector.tensor_copy(ROp[C:C + 1, :], RO[C:C + 1, :])

            if debug_out:
                nc.sync.dma_start(out=dbg_idx[:, :], in_=idxall)
                nc.sync.dma_start(out=dbg_sq[:, :], in_=C4[3:4, :])
                nc.sync.dma_start(out=dbg_stats[0:1, :], in_=stats_sb)
                nc.sync.dma_start(out=dbg_stats[1:2, :], in_=stats2)
                nc.sync.dma_start(out=dbg_sv[0:1, :], in_=svec)
                nc.sync.dma_start(out=dbg_sv[1:2, :], in_=tvec)
                nc.sync.dma_start(out=dbg_sv[2:3, 0:C], in_=srep[100:101, :])
                nc.sync.dma_start(out=dbg_sv[3:4, 0:C], in_=w2p[100:101, :])
                nc.sync.dma_start(out=dbg_sv[4:5, 0:C].rearrange("o (p q) -> p (o q)", q=1), in_=sv64[:, 0:1])
                nc.sync.dma_start(out=dbg_sv[5:6, :], in_=tps_row)
            t128_ps = cnps.tile([128, 2 * C], F32, tag="cps")
            nc.tensor.matmul(t128_ps, lhsT=ones1, rhs=tps_row, start=True, stop=True)
            t128 = sing.tile([128, 2 * C], F32)
            nc.scalar.copy(t128, t128_ps)
            ostash_d = dram.tile([C, NQ], F32)
            osums = sing.tile([C, NT], F32)
            osums2 = sing.tile([C, NT], F32)

            # ---------------- P2: attention + value + output proj ----------------
            for t in range(NT):
                qs = slice(t * 128, (t + 1) * 128)
                G2 = gp.tile([128, K, 2 * C], F32, tag="g")
                nc.sync.dma_start(out=G2,
                                  in_=hv_d[t * 128:(t + 1) * 128, :].rearrange(
                                      "p (k c) -> p k c", k=K))
                getattr(nc, stt_engine).scalar_tensor_tensor(
                    out=G2, in0=G2, scalar=0.0, in1=_b(t128[:, :], 1, K),
                    op0=ALU.bypass, op1=ALU.add)
                HVn = scrp.tile([128, K * 2 * C], F32, tag="hvn")
                nc.scalar.activation(HVn, G2.rearrange("p k c -> p (k c)"), ACTF.Relu)
                HVn3 = HVn.rearrange("p (k c) -> p k c", k=K)
                # logits & softmax over K
                lsc = scrp.tile([128, K, H], F32, tag="lsc")
                getattr(nc, stt_engine).scalar_tensor_tensor(
                    out=lsc, in0=HVn3[:, :, 0:C], scalar=0.0,
                    in1=_b(w2p[:, :], 1, K), op0=ALU.bypass, op1=ALU.mult)
                logit = smp.tile([128, K], F32)
                nc.vector.tensor_reduce(out=logit, in_=lsc, axis=AX.X, op=ALU.add)
                mx = smp.tile([128, 1], F32)
                nc.vector.tensor_reduce(out=mx, in_=logit, axis=AX.X, op=ALU.max)
                nc.vector.tensor_scalar_mul(mx, mx, -1.0)
                ex = smp.tile([128, K], F32)
                nc.scalar.activation(ex, logit, ACTF.Exp, bias=mx[:, 0:1])
                sume = smp.tile([128, 1], F32)
                nc.vector.tensor_reduce(out=sume, in_=ex, axis=AX.X, op=ALU.add)
                rec = smp.tile([128, 1], F32)
                nc.vector.reciprocal(rec, sume)
                attn = smp.tile([128, K], F32)
                nc.vector.tensor_scalar_mul(attn, ex, rec[:, 0:1])
                # weighted sum over K
                prod = scrp.tile([128, K, C], F32, tag="lsc")
                getattr(nc, stt_engine).scalar_tensor_tensor(
                    out=prod, in0=HVn3[:, :, C:2 * C], scalar=0.0,
                    in1=_b(attn[:, :], 2, C), op0=ALU.bypass, op1=ALU.mult)
                outq = smp.tile([128, C], F32, tag="outq")
                nc.vector.tensor_reduce(out=outq, in_=prod.rearrange("p k c -> p c k"),
                                        axis=AX.X, op=ALU.add)
                if debug_out and t == 0:
                    nc.sync.dma_start(out=dbg_p2[:, 0:K * 2 * C], in_=HVn)
                    nc.sync.dma_start(out=dbg_p2[:, K * 2 * C:K * 2 * C + K], in_=logit)
                    nc.sync.dma_start(out=dbg_p2[:, K * 2 * C + K:K * 2 * C + 2 * K], in_=attn)
                    nc.sync.dma_start(out=dbg_p2[:, K * 2 * C + 2 * K:K * 2 * C + 2 * K + C], in_=outq)
                    nc.sync.dma_start(out=dbg_p2[:, K * 2 * C + 2 * K + C:], in_=t128)
                # o = (out @ Wo.T * sv) + bo, via transpose + matmul
                tps = trps.tile([C, 128], F32)
                nc.tensor.transpose(tps, outq, ident)
                ot5 = otp_sb.tile([C + 1, 128], F32)
                nc.vector.memset(ot5[C:C + 1, :], 1.0)
                nc.scalar.copy(ot5[0:C, :], tps)
                ops_ = otps.tile([C, 128], F32)
                nc.tensor.matmul(ops_, lhsT=ROp, rhs=ot5, start=True, stop=True)
                osb = otp_sb.tile([C, 128], F32, tag="osb")
                nc.scalar.activation(osb, ops_, ACTF.Copy,
                                     accum_out=osums[:, t:t + 1])
                nc.sync.dma_start(out=ostash_d[:, qs], in_=osb)
                osq = otp_sb.tile([C, 128], F32, tag="osq")
                nc.scalar.activation(osq, ops_, ACTF.Square,
                                     accum_out=osums2[:, t:t + 1])

            # ---------------- AR2 ----------------
            ost = sing.tile([C, 2], F32)
            nc.vector.tensor_reduce(out=ost[:, 0:1], in_=osums, axis=AX.X, op=ALU.add)
            nc.vector.tensor_reduce(out=ost[:, 1:2], in_=osums2, axis=AX.X, op=ALU.add)
            bi2 = dram.tile([C, 2], F32)
            bo2 = dram.tile([C, 2], F32)
            nc.sync.dma_start(out=bi2, in_=ost)
            if n_cores > 1:
                nc.gpsimd.collective_compute(
                    "AllReduce", ALU.add,
                    replica_groups=[list(range(n_cores))],
                    ins=[bi2[:, :].opt()], outs=[bo2[:, :].opt()])
            else:
                nc.sync.dma_start(out=bo2[:, :], in_=bi2[:, :])
            ost2 = sing.tile([C, 2], F32)
            nc.sync.dma_start(out=ost2, in_=bo2)
            omean = sing.tile([C, 1], F32)
            nc.vector.tensor_scalar_mul(omean, ost2[:, 0:1], 1.0 / tot_pts)
            ovar = sing.tile([C, 1], F32)
            nc.vector.tensor_scalar_mul(ovar, ost2[:, 1:2], 1.0 / tot_pts)
            omsq = sing.tile([C, 1], F32)
            nc.vector.tensor_mul(omsq, omean, omean)
            nc.vector.tensor_sub(ovar, ovar, omsq)
            nc.vector.tensor_scalar_add(ovar, ovar, EPS)
            osd = sing.tile([C, 1], F32)
            nc.scalar.sqrt(osd, ovar)
            orst = sing.tile([C, 1], F32)
            nc.vector.reciprocal(orst, osd)
            so = sing.tile([C, 1], F32)
            nc.vector.tensor_mul(so, gbo[:, 0:1], orst)
            to = sing.tile([C, 1], F32)
            nc.vector.tensor_mul(to, omean, so)
            nc.vector.tensor_sub(to, gbo[:, 1:2], to)

            if debug_out:
                nc.sync.dma_start(out=dbg_ot[:, 0:1], in_=so)
                nc.sync.dma_start(out=dbg_ot[:, 1:2], in_=to)
                nc.sync.dma_start(out=dbg_ot[:, 2:3], in_=ost[:, 0:1])
                nc.sync.dma_start(out=dbg_ot[:, 3:4], in_=ost2[:, 0:1])
            # ---------------- P3: BN + relu + residual ----------------
            P3CH = min(1024, NQ)
            for j in range(NQ // P3CH):
                js = slice(j * P3CH, (j + 1) * P3CH)
                ob = scrp.tile([C, P3CH], F32, tag="hvn")
                nc.sync.dma_start(out=ob, in_=ostash_d[:, js])
                nc.scalar.activation(ob, ob, ACTF.Relu, bias=to[:, 0:1],
                                     scale=so[:, 0:1])
                nc.vector.scalar_tensor_tensor(out=ob, in0=ob, scalar=0.0,
                                               in1=F_sbq[:, js],
                                               op0=ALU.bypass, op1=ALU.add)
                nc.sync.dma_start(out=outd[:, js], in_=ob)

    nc.compile()
    return nc


def make_in_maps(xyz, feats, W1, b1, g1, be1, W2, b2, Wv, bv, gv, bev,
                 Wo, bo, go, beo, n_cores=8, N=8192, NQ=4096):
    """Shard/lay out the full inputs into per-core input dicts (layout only)."""
    f32 = np.float32
    W1 = np.asarray(W1, f32)
    Wv = np.asarray(Wv, f32)
    # RAB: rows [in-ch(64); xyz(3)], cols [A(64) | Bv(64)]
    RAB = np.concatenate([
        np.concatenate([W1[:, C:2 * C].T, W1[:, 2 * C:2 * C + 3].T], axis=0),
        np.concatenate([Wv[:, 0:C].T, Wv[:, C:C + 3].T], axis=0),
    ], axis=1).astype(f32)
    RCN = np.concatenate([
        np.concatenate([W1[:, 0:C].T, W1[:, 2 * C:2 * C + 3].T,
                        np.asarray(b1, f32)[None, :]], axis=0),
        np.concatenate([np.zeros((C, C), f32), Wv[:, C:C + 3].T,
                        np.asarray(bv, f32)[None, :]], axis=0),
    ], axis=1).astype(f32)
    w2rep = np.ascontiguousarray(np.broadcast_to(np.asarray(W2, f32)[0], (128, H)))
    gbp = np.stack([np.concatenate([np.asarray(g1, f32), np.asarray(gv, f32)]),
                    np.concatenate([np.asarray(be1, f32), np.asarray(bev, f32)])])
    RO = np.concatenate([np.asarray(Wo, f32).T, np.asarray(bo, f32)[None, :]], axis=0)
    gbo = np.stack([np.asarray(go, f32), np.asarray(beo, f32)], axis=1)

    xyz = np.asarray(xyz, f32)
    feats = np.asarray(feats, f32)
    halves = n_cores // xyz.shape[0]      # cores per batch element
    in_maps = []
    for c in range(n_cores):
        b = c // halves
        h = c % halves
        xb = np.roll(xyz[b], -h * NQ, axis=1)
        fb = np.roll(feats[b], -h * NQ, axis=1)
        in_maps.append({
            "xyzc": np.ascontiguousarray(xb),
            "xyzq": np.ascontiguousarray(
                np.concatenate([xb[:, 0:NQ], -np.ones((1, NQ), f32)], axis=0)),
            "xyzT": np.ascontiguousarray(xb.T),
            "fc": np.ascontiguousarray(fb),
            "RAB": RAB, "RCN": RCN, "w2rep": w2rep, "gb": gbp,
            "RO": np.ascontiguousarray(RO), "gbo": np.ascontiguousarray(gbo),
        })
    return in_maps


_NC_CACHE = {}


def kernel(**inputs):
    from concourse.bass_utils import run_bass_kernel_spmd
    B, _, N = inputs["xyz"].shape
    n_cores = 8
    NQ = N * B // n_cores
    key = (N, NQ, n_cores)
    if key not in _NC_CACHE:
        _NC_CACHE[key] = build_nc(N=N, NQ=NQ, n_cores=n_cores)
    nc = _NC_CACHE[key]
    in_maps = make_in_maps(n_cores=n_cores, N=N, NQ=NQ, **inputs)
    res = run_bass_kernel_spmd(nc, in_maps, core_ids=list(range(n_cores)))
    halves = n_cores // B
    out = np.empty((B, C, N), np.float32)
    for c in range(n_cores):
        b, h = c // halves, c % halves
        out[b][:, h * NQ:(h + 1) * NQ] = res.results[c]["out"]
    return out

